# revision 8
# baseline (speedup 1.0000x reference)
"""AttentionAutoInt Trainium2 kernel (8-core data-parallel).

reference:
    q,k,v,r = x@Wq, x@Wk, x@Wv, x@Wr        (per-field shared projections)
    scores  = q @ k^T  per sample           ([64,64], softmax over last axis)
    out     = relu(r + softmax(scores) @ v)

Math restructure used here:
    scores = x @ A @ x^T with A = Wq @ Wk^T (precomputed on host), which
    saves one of the four projections.  Per sample, with everything kept in
    a transposed "feature-on-partitions" layout:
        xT  = x^T                    (PE transpose)
        cT  = A^T @ xT               (f32r matmul, N=512)
        v,r = xT^T @ [Wv|Wr]         (f32r matmul, normal layout, N=256)
        sT  = xT^T(slice) @ cT(slice)   per sample -> scoresT [n, m]
        e   = exp(sT)                (ACT, no max-subtraction: |s| <~ 70)
        U   = e^T @ [v|1]            (bf16 matmul; ones column gives rowsum)
        out = relu(r + U * (1/rowsum))   (AFFINE_THEN_ADD + relu)

Sharding: batch B=8192 split across 8 cores (1024 samples = 65536 tokens
per core), weights replicated; no cross-core communication.
"""

import sys

for _p in ("/opt/trn_rl_repo", "/root/.axon_site/_ro/trn_rl_repo"):
    if _p not in sys.path:
        sys.path.append(_p)

import numpy as np

B, M, D, DP = 8192, 64, 128, 128
NCORES = 8
BC = B // NCORES          # samples per core
TOK = BC * M              # tokens per core = 65536
TILE = 512                # tokens per pipeline tile
NBLK = TILE // 128        # 128-token blocks (= sample pairs) per tile
NT_FULL = TOK // TILE     # 128 tiles per core

_BUILD_CACHE: dict = {}


def build(ntiles=NT_FULL, num_devices=NCORES):
    """Build the Bass module. One core processes ntiles*512 tokens."""
    key = (ntiles, num_devices)
    if key in _BUILD_CACHE:
        return _BUILD_CACHE[key]

    from contextlib import ExitStack

    import concourse.bacc as bacc
    import concourse.mybir as mybir
    import concourse.tile as tile

    f32 = mybir.dt.float32
    f32r = mybir.dt.float32r
    bf16 = mybir.dt.bfloat16
    Exp = mybir.ActivationFunctionType.Exp

    tok = ntiles * TILE
    nc = bacc.Bacc(
        "TRN2", target_bir_lowering=False, debug=False, num_devices=num_devices
    )
    x_d = nc.dram_tensor("x", [tok, D], f32, kind="ExternalInput").ap()
    a_d = nc.dram_tensor("Amat", [D, DP], f32r, kind="ExternalInput").ap()
    wvr_d = nc.dram_tensor("Wvr", [D, 2 * DP], f32r, kind="ExternalInput").ap()
    id_d = nc.dram_tensor("ident", [128, 128], f32, kind="ExternalInput").ap()
    out_d = nc.dram_tensor("out", [tok, DP], f32, kind="ExternalOutput").ap()

    with tile.TileContext(nc) as tc, ExitStack() as ctx:
        P = lambda name, bufs, **kw: ctx.enter_context(
            tc.tile_pool(name=name, bufs=bufs, **kw)
        )
        consts = P("consts", 1)
        xpool = P("x", 4)
        xtpool = P("xt", 2)
        ctpool = P("ct", 2)
        vpool = P("v", 2)
        rpool = P("r", 2)
        epool = P("e", 2)
        rcpool = P("rc", 3)
        opool = P("o", 4)
        # PSUM: 8 banks total; these add up to exactly 8.
        xt_ps_p = P("xtp", 1, space="PSUM")   # [128,512] f32 = 1 bank
        ct_ps_p = P("ctp", 2, space="PSUM")   # 2 banks
        vr_ps_p = P("vrp", 1, space="PSUM")   # [128,4,256] f32 = 2 banks
        sc_ps_p = P("scp", 1, space="PSUM")   # [128,4,64] f32 -> 1 bank
        u_ps_p = P("up", 2, space="PSUM")     # [128,2,132] f32 -> 1 bank x2

        a_sb = consts.tile([D, DP], f32r)
        nc.sync.dma_start(a_sb[:], a_d[:])
        wvr_sb = consts.tile([D, 2 * DP], f32r)
        nc.sync.dma_start(wvr_sb[:], wvr_d[:])
        id_sb = consts.tile([128, 128], f32)
        nc.sync.dma_start(id_sb[:], id_d[:])

        for t in range(ntiles):
            # ---- load x tile: 512 tokens as 4 blocks of 128 on partitions
            x_sb = xpool.tile([128, NBLK, D], f32)
            nc.sync.dma_start(
                x_sb[:],
                x_d[t * TILE : (t + 1) * TILE, :].rearrange(
                    "(b p) d -> p b d", p=128
                ),
            )

            # ---- transpose to xT [d, tok] (PE), copy to SBUF (DVE)
            xt_ps = xt_ps_p.tile([128, TILE], f32)
            for b in range(NBLK):
                nc.tensor.transpose(
                    xt_ps[:, b * 128 : (b + 1) * 128], x_sb[:, b, :], id_sb[:]
                )
            xt_sb = xtpool.tile([128, TILE], f32r)
            nc.vector.tensor_copy(xt_sb[:], xt_ps[:])

            # ---- cT = A^T @ xT   (f32r, N=512)
            ct_ps = ct_ps_p.tile([128, TILE], f32)
            nc.tensor.matmul(
                ct_ps[:],
                a_sb[:],
                xt_sb[:],
                start=True,
                stop=True,
            )
            ct_sb = ctpool.tile([128, TILE], f32)
            nc.scalar.copy(ct_sb[:], ct_ps[:])

            # ---- [v|r] = x @ [Wv|Wr] per block (f32r, N=256, normal layout)
            vr_ps = vr_ps_p.tile([128, NBLK, 2 * DP], f32)
            for b in range(NBLK):
                nc.tensor.matmul(
                    vr_ps[:, b, :],
                    xt_sb[:, b * 128 : (b + 1) * 128],
                    wvr_sb[:],
                    start=True,
                    stop=True,
                )
            v_bf = vpool.tile([128, NBLK, 132], bf16)
            nc.scalar.copy(v_bf[:, :, 0:128], vr_ps[:, :, 0:128])
            nc.gpsimd.memset(v_bf[:, :, 128:129], 1.0)
            r_bf = rpool.tile([128, NBLK, DP], bf16)
            nc.scalar.copy(r_bf[:], vr_ps[:, :, 128:256])

            # ---- scoresT per sample pair (fp32, col-tiled concurrent MMs)
            sc_ps = sc_ps_p.tile([128, NBLK, M], f32)
            for p in range(NBLK):
                c0 = p * 128
                nc.tensor.matmul(
                    sc_ps[0:64, p, :],
                    xt_sb[:, c0 : c0 + 64].bitcast(f32),
                    ct_sb[:, c0 : c0 + 64],
                    start=True,
                    stop=True,
                    tile_position=(0, 0),
                )
                nc.tensor.matmul(
                    sc_ps[64:128, p, :],
                    xt_sb[:, c0 + 64 : c0 + 128].bitcast(f32),
                    ct_sb[:, c0 + 64 : c0 + 128],
                    start=True,
                    stop=True,
                    tile_position=(0, 64),
                )
            exp_bf = epool.tile([128, NBLK, M], bf16)
            nc.scalar.activation(exp_bf[:], sc_ps[:], Exp)

            # ---- U = exp^T @ [v|1] per sample (bf16, diagonal-packed pairs)
            out_sb = opool.tile([128, NBLK, DP], f32)
            for h in range(2):
                u_ps = u_ps_p.tile([128, 2, 132], f32)
                for pp in range(2):
                    p = h * 2 + pp
                    nc.tensor.matmul(
                        u_ps[0:64, pp, 0:129],
                        exp_bf[0:64, p, :],
                        v_bf[0:64, p, 0:129],
                        start=True,
                        stop=True,
                        tile_position=(0, 0),
                    )
                    nc.tensor.matmul(
                        u_ps[64:128, pp, 0:129],
                        exp_bf[64:128, p, :],
                        v_bf[64:128, p, 0:129],
                        start=True,
                        stop=True,
                        tile_position=(64, 64),
                    )
                recip = rcpool.tile([128, 2, 1], f32)
                nc.vector.reciprocal(recip[:], u_ps[:, :, 128:129])
                for pp in range(2):
                    p = h * 2 + pp
                    # out = (U * (1/rowsum) + 0) + r
                    nc.vector.affine_then_add(
                        out_sb[:, p, :],
                        u_ps[:, pp, 0:128],
                        r_bf[:, p, :],
                        scale=recip[:, pp, :],
                        bias=0.0,
                    )
            # relu in place (gpsimd; SBUF->SBUF)
            nc.gpsimd.tensor_scalar_max(out_sb[:], out_sb[:], 0.0)

            nc.scalar.dma_start(
                out_d[t * TILE : (t + 1) * TILE, :].rearrange(
                    "(b p) d -> p b d", p=128
                ),
                out_sb[:],
            )

    nc.finalize()
    _BUILD_CACHE[key] = nc
    return nc


def make_inputs(x_shard, Wq, Wk, Wv, Wr):
    """Per-core input map from a token-flattened x shard [tok, D]."""
    A = (Wq.astype(np.float64) @ Wk.astype(np.float64).T).astype(np.float32)
    Wvr = np.concatenate([Wv, Wr], axis=1).astype(np.float32)
    ident = np.eye(128, dtype=np.float32)
    return {
        "x": np.ascontiguousarray(x_shard, dtype=np.float32),
        "Amat": A,
        "Wvr": Wvr,
        "ident": ident,
    }


def run(inputs, trace=False):
    """Run on 8 cores; returns (output [B,M,D], BassKernelResults)."""
    from concourse.bass_utils import run_bass_kernel_spmd

    x = np.asarray(inputs["x"], dtype=np.float32)
    Wq = np.asarray(inputs["Wq"], dtype=np.float32)
    Wk = np.asarray(inputs["Wk"], dtype=np.float32)
    Wv = np.asarray(inputs["Wv"], dtype=np.float32)
    Wr = np.asarray(inputs["Wr"], dtype=np.float32)

    nc = build()
    x_flat = x.reshape(NCORES, TOK, D)
    in_maps = [make_inputs(x_flat[i], Wq, Wk, Wv, Wr) for i in range(NCORES)]
    res = run_bass_kernel_spmd(nc, in_maps, list(range(NCORES)), trace=trace)
    out = np.stack([res.results[i]["out"] for i in range(NCORES)], axis=0)
    return out.reshape(B, M, DP), res


def kernel(x, Wq, Wk, Wv, Wr):
    out, _ = run({"x": x, "Wq": Wq, "Wk": Wk, "Wv": Wv, "Wr": Wr}, trace=False)
    return out


# revision 13
# speedup vs baseline: 2.5463x; 2.5463x over previous
"""AttentionAutoInt Trainium2 kernel (8-core data-parallel).

reference:
    q,k,v,r = x@Wq, x@Wk, x@Wv, x@Wr        (per-field shared projections)
    scores  = q @ k^T  per sample           ([64,64], softmax over last axis)
    out     = relu(r + softmax(scores) @ v)

Math restructure used here:
    scores = x @ A @ x^T with A = Wq @ Wk^T (precomputed on host), which
    saves one of the four projections.  Per sample, with everything kept in
    a transposed "feature-on-partitions" layout:
        xT  = x^T                    (PE transpose)
        cT  = A^T @ xT               (f32r matmul, N=512)
        v,r = xT^T @ [Wv|Wr]         (f32r matmul, normal layout, N=256)
        sT  = xT^T(slice) @ cT(slice)   per sample -> scoresT [n, m]
        e   = exp(sT)                (ACT, no max-subtraction: |s| <~ 70)
        U   = e^T @ [v|1]            (bf16 matmul; ones column gives rowsum)
        out = relu(r + U * (1/rowsum))   (AFFINE_THEN_ADD + relu)

Sharding: batch B=8192 split across 8 cores (1024 samples = 65536 tokens
per core), weights replicated; no cross-core communication.
"""

import sys

for _p in ("/opt/trn_rl_repo", "/root/.axon_site/_ro/trn_rl_repo"):
    if _p not in sys.path:
        sys.path.append(_p)

import numpy as np

B, M, D, DP = 8192, 64, 128, 128
NCORES = 8
BC = B // NCORES          # samples per core
TOK = BC * M              # tokens per core = 65536
TILE = 512                # tokens per pipeline tile
NBLK = TILE // 128        # 128-token blocks (= sample pairs) per tile
NT_FULL = TOK // TILE     # 128 tiles per core

_BUILD_CACHE: dict = {}


def build(ntiles=NT_FULL, num_devices=NCORES):
    """Build the Bass module. One core processes ntiles*512 tokens."""
    key = (ntiles, num_devices)
    if key in _BUILD_CACHE:
        return _BUILD_CACHE[key]

    from contextlib import ExitStack

    import concourse.bacc as bacc
    import concourse.mybir as mybir
    import concourse.tile as tile

    f32 = mybir.dt.float32
    f32r = mybir.dt.float32r
    bf16 = mybir.dt.bfloat16
    Exp = mybir.ActivationFunctionType.Exp

    tok = ntiles * TILE
    nc = bacc.Bacc(
        "TRN2", target_bir_lowering=False, debug=False, num_devices=num_devices
    )
    x_d = nc.dram_tensor("x", [tok, D], f32, kind="ExternalInput").ap()
    a_d = nc.dram_tensor("Amat", [D, DP], f32r, kind="ExternalInput").ap()
    wvr_d = nc.dram_tensor("Wvr", [D, 2 * DP], f32r, kind="ExternalInput").ap()
    id_d = nc.dram_tensor("ident", [128, 128], f32, kind="ExternalInput").ap()
    out_d = nc.dram_tensor("out", [tok, DP], f32, kind="ExternalOutput").ap()

    with tile.TileContext(nc) as tc, ExitStack() as ctx:
        P = lambda name, bufs, **kw: ctx.enter_context(
            tc.tile_pool(name=name, bufs=bufs, **kw)
        )
        consts = P("consts", 1)
        xpool = P("x", 4)
        xtpool = P("xt", 2)
        ctpool = P("ct", 2)
        vpool = P("v", 2)
        rpool = P("r", 2)
        epool = P("e", 2)
        rcpool = P("rc", 3)
        opool = P("o", 4)
        # PSUM: 8 banks total; these add up to exactly 8.
        xt_ps_p = P("xtp", 1, space="PSUM")   # [128,512] f32 = 1 bank
        ct_ps_p = P("ctp", 2, space="PSUM")   # 2 banks
        vr_ps_p = P("vrp", 1, space="PSUM")   # [128,4,256] f32 = 2 banks
        sc_ps_p = P("scp", 1, space="PSUM")   # [128,4,64] f32 -> 1 bank
        u_ps_p = P("up", 2, space="PSUM")     # [128,2,132] f32 -> 1 bank x2

        a_sb = consts.tile([D, DP], f32r)
        nc.sync.dma_start(a_sb[:], a_d[:])
        wvr_sb = consts.tile([D, 2 * DP], f32r)
        nc.sync.dma_start(wvr_sb[:], wvr_d[:])
        id_sb = consts.tile([128, 128], f32)
        nc.sync.dma_start(id_sb[:], id_d[:])

        for t in range(ntiles):
            # ---- load x tile: 512 tokens as 4 blocks of 128 on partitions
            x_sb = xpool.tile([128, NBLK, D], f32)
            nc.sync.dma_start(
                x_sb[:],
                x_d[t * TILE : (t + 1) * TILE, :].rearrange(
                    "(b p) d -> p b d", p=128
                ),
            )

            # ---- transpose to xT [d, tok] (PE), copy to SBUF (DVE)
            xt_ps = xt_ps_p.tile([128, TILE], f32)
            for b in range(NBLK):
                nc.tensor.transpose(
                    xt_ps[:, b * 128 : (b + 1) * 128], x_sb[:, b, :], id_sb[:]
                )
            xt_sb = xtpool.tile([128, TILE], f32r)
            nc.vector.tensor_copy(xt_sb[:], xt_ps[:])

            # ---- cT = A^T @ xT   (f32r, N=512)
            ct_ps = ct_ps_p.tile([128, TILE], f32)
            nc.tensor.matmul(
                ct_ps[:],
                a_sb[:],
                xt_sb[:],
                start=True,
                stop=True,
            )
            ct_sb = ctpool.tile([128, TILE], f32r)
            nc.scalar.copy(ct_sb[:], ct_ps[:])

            # ---- [v|r] = x @ [Wv|Wr] per block (f32r, N=256, normal layout)
            vr_ps = vr_ps_p.tile([128, NBLK, 2 * DP], f32)
            for b in range(NBLK):
                nc.tensor.matmul(
                    vr_ps[:, b, :],
                    xt_sb[:, b * 128 : (b + 1) * 128],
                    wvr_sb[:],
                    start=True,
                    stop=True,
                )
            v_bf = vpool.tile([128, NBLK, 132], bf16)
            nc.scalar.copy(v_bf[:, :, 0:128], vr_ps[:, :, 0:128])
            nc.gpsimd.memset(v_bf[:, :, 128:129], 1.0)
            r_bf = rpool.tile([128, NBLK, DP], bf16)
            nc.scalar.copy(r_bf[:], vr_ps[:, :, 128:256])

            # ---- scoresT per sample pair (f32r, full 128x128 blocks; the
            # cross-sample quadrants are garbage and never read by U)
            sc_ps = sc_ps_p.tile([128, NBLK, 2 * M], f32)
            for p in range(NBLK):
                c0 = p * 128
                nc.tensor.matmul(
                    sc_ps[:, p, :],
                    xt_sb[:, c0 : c0 + 128],
                    ct_sb[:, c0 : c0 + 128],
                    start=True,
                    stop=True,
                )
            exp_bf = epool.tile([128, NBLK, 2 * M], bf16)
            nc.scalar.activation(exp_bf[:], sc_ps[:], Exp)

            # ---- U = exp^T @ [v|1] per sample (bf16, diagonal-packed pairs)
            out_sb = opool.tile([128, NBLK, DP], f32)
            for h in range(2):
                u_ps = u_ps_p.tile([128, 2, 132], f32)
                for pp in range(2):
                    p = h * 2 + pp
                    nc.tensor.matmul(
                        u_ps[0:64, pp, 0:129],
                        exp_bf[0:64, p, 0:64],
                        v_bf[0:64, p, 0:129],
                        start=True,
                        stop=True,
                        tile_position=(0, 0),
                    )
                    nc.tensor.matmul(
                        u_ps[64:128, pp, 0:129],
                        exp_bf[64:128, p, 64:128],
                        v_bf[64:128, p, 0:129],
                        start=True,
                        stop=True,
                        tile_position=(64, 64),
                    )
                recip = rcpool.tile([128, 2, 1], f32)
                nc.vector.reciprocal(recip[:], u_ps[:, :, 128:129])
                for pp in range(2):
                    p = h * 2 + pp
                    # out = (U * (1/rowsum) + 0) + r
                    nc.vector.affine_then_add(
                        out_sb[:, p, :],
                        u_ps[:, pp, 0:128],
                        r_bf[:, p, :],
                        scale=recip[:, pp, :],
                        bias=0.0,
                    )
            # relu in place (gpsimd; SBUF->SBUF)
            nc.vector.tensor_scalar_max(out_sb[:], out_sb[:], 0.0)

            nc.scalar.dma_start(
                out_d[t * TILE : (t + 1) * TILE, :].rearrange(
                    "(b p) d -> p b d", p=128
                ),
                out_sb[:],
            )

    nc.finalize()
    _BUILD_CACHE[key] = nc
    return nc


def make_inputs(x_shard, Wq, Wk, Wv, Wr):
    """Per-core input map from a token-flattened x shard [tok, D]."""
    A = (Wq.astype(np.float64) @ Wk.astype(np.float64).T).astype(np.float32)
    Wvr = np.concatenate([Wv, Wr], axis=1).astype(np.float32)
    ident = np.eye(128, dtype=np.float32)
    return {
        "x": np.ascontiguousarray(x_shard, dtype=np.float32),
        "Amat": A,
        "Wvr": Wvr,
        "ident": ident,
    }


def run(inputs, trace=False):
    """Run on 8 cores; returns (output [B,M,D], BassKernelResults)."""
    from concourse.bass_utils import run_bass_kernel_spmd

    x = np.asarray(inputs["x"], dtype=np.float32)
    Wq = np.asarray(inputs["Wq"], dtype=np.float32)
    Wk = np.asarray(inputs["Wk"], dtype=np.float32)
    Wv = np.asarray(inputs["Wv"], dtype=np.float32)
    Wr = np.asarray(inputs["Wr"], dtype=np.float32)

    nc = build()
    x_flat = x.reshape(NCORES, TOK, D)
    in_maps = [make_inputs(x_flat[i], Wq, Wk, Wv, Wr) for i in range(NCORES)]
    res = run_bass_kernel_spmd(nc, in_maps, list(range(NCORES)), trace=trace)
    out = np.stack([res.results[i]["out"] for i in range(NCORES)], axis=0)
    return out.reshape(B, M, DP), res


def kernel(x, Wq, Wk, Wv, Wr):
    out, _ = run({"x": x, "Wq": Wq, "Wk": Wk, "Wv": Wv, "Wr": Wr}, trace=False)
    return out


# revision 16
# speedup vs baseline: 3.0920x; 1.2143x over previous
"""AttentionAutoInt Trainium2 kernel (8-core data-parallel).

reference:
    q,k,v,r = x@Wq, x@Wk, x@Wv, x@Wr        (per-field shared projections)
    scores  = q @ k^T  per sample           ([64,64], softmax over last axis)
    out     = relu(r + softmax(scores) @ v)

Math restructure used here:
    scores = x @ A @ x^T with A = Wq @ Wk^T (precomputed on host), which
    saves one of the four projections.  Per sample, with everything kept in
    a transposed "feature-on-partitions" layout:
        xT  = x^T                    (PE transpose)
        cT  = A^T @ xT               (f32r matmul, N=512)
        v,r = xT^T @ [Wv|Wr]         (f32r matmul, normal layout, N=256)
        sT  = xT^T(slice) @ cT(slice)   per sample -> scoresT [n, m]
        e   = exp(sT)                (ACT, no max-subtraction: |s| <~ 70)
        U   = e^T @ [v|1]            (bf16 matmul; ones column gives rowsum)
        out = relu(r + U * (1/rowsum))   (AFFINE_THEN_ADD + relu)

Sharding: batch B=8192 split across 8 cores (1024 samples = 65536 tokens
per core), weights replicated; no cross-core communication.
"""

import sys

for _p in ("/opt/trn_rl_repo", "/root/.axon_site/_ro/trn_rl_repo"):
    if _p not in sys.path:
        sys.path.append(_p)

import numpy as np

B, M, D, DP = 8192, 64, 128, 128
NCORES = 8
BC = B // NCORES          # samples per core
TOK = BC * M              # tokens per core = 65536
TILE = 512                # tokens per pipeline tile
NBLK = TILE // 128        # 128-token blocks (= sample pairs) per tile
NT_FULL = TOK // TILE     # 128 tiles per core

_BUILD_CACHE: dict = {}


def _get_relu_affine():
    """Register (once) a fused DVE op: out = relu((in0*s0 + s1) + in1)."""
    import concourse.dve_ops as dve_ops
    from concourse.dve_spec import C0, C1, Src0, Src1, Spec, lower, relu
    from concourse.dve_uop import DveOpSpec

    name = "RELU_AFFINE_ANT"
    for op in dve_ops.OPS:
        if op.name == name:
            return op
    spec = Spec(
        body=relu((Src0 * C0 + C1) + Src1),
        reference=lambda in0, in1, s0, s1, imm2: np.maximum(
            (in0.astype(np.float32) * s0 + s1) + in1.astype(np.float32), 0.0
        ),
    )
    row = max(dve_ops._SUB_OPCODE_FOR_NAME.values()) + 1
    assert row < 0x20
    dve_ops._SUB_OPCODE_FOR_NAME[name] = row
    shas = {}
    for ver in ("v3", "v4"):
        try:
            u = lower(spec, ver=ver)
            shas[ver] = DveOpSpec(name=name, opcode=row, uops=u, rd1_en=True).sha(ver)
        except Exception:
            pass
    op = dve_ops.DveOp(name, spec, subdim=False, uops_sha=shas)
    dve_ops.OPS.append(op)
    dve_ops.CUSTOM_DVE_SPECS[name] = spec
    return op


def build(ntiles=NT_FULL, num_devices=NCORES):
    """Build the Bass module. One core processes ntiles*512 tokens."""
    key = (ntiles, num_devices)
    if key in _BUILD_CACHE:
        return _BUILD_CACHE[key]

    from contextlib import ExitStack

    import concourse.bacc as bacc
    import concourse.mybir as mybir
    import concourse.tile as tile

    f32 = mybir.dt.float32
    f32r = mybir.dt.float32r
    bf16 = mybir.dt.bfloat16
    Exp = mybir.ActivationFunctionType.Exp

    relu_affine = _get_relu_affine()

    tok = ntiles * TILE
    nc = bacc.Bacc(
        "TRN2", target_bir_lowering=False, debug=False, num_devices=num_devices
    )
    x_d = nc.dram_tensor("x", [tok, D], f32, kind="ExternalInput").ap()
    a_d = nc.dram_tensor("Amat", [D, DP], f32r, kind="ExternalInput").ap()
    wvr_d = nc.dram_tensor("Wvr", [D, 2 * DP], f32r, kind="ExternalInput").ap()
    id_d = nc.dram_tensor("ident", [128, 128], f32, kind="ExternalInput").ap()
    out_d = nc.dram_tensor("out", [tok, DP], f32, kind="ExternalOutput").ap()

    with tile.TileContext(nc) as tc, ExitStack() as ctx:
        P = lambda name, bufs, **kw: ctx.enter_context(
            tc.tile_pool(name=name, bufs=bufs, **kw)
        )
        consts = P("consts", 1)
        xpool = P("x", 4)
        xtpool = P("xt", 2)
        ctpool = P("ct", 2)
        vpool = P("v", 2)
        epool = P("e", 2)
        rcpool = P("rc", 3)
        opool = P("o", 4)
        # PSUM: 8 banks total; these add up to exactly 8.
        xt_ps_p = P("xtp", 1, space="PSUM")   # [128,512] f32 = 1 bank
        ct_ps_p = P("ctp", 2, space="PSUM")   # 2 banks
        vr_ps_p = P("vrp", 1, space="PSUM")   # [128,4,256] f32 = 2 banks
        sc_ps_p = P("scp", 1, space="PSUM")   # [128,4,64] f32 -> 1 bank
        u_ps_p = P("up", 2, space="PSUM")     # [128,2,132] f32 -> 1 bank x2

        a_sb = consts.tile([D, DP], f32r)
        nc.sync.dma_start(a_sb[:], a_d[:])
        wvr_sb = consts.tile([D, 2 * DP], f32r)
        nc.sync.dma_start(wvr_sb[:], wvr_d[:])
        id_sb = consts.tile([128, 128], f32)
        nc.sync.dma_start(id_sb[:], id_d[:])

        for t in range(ntiles):
            # ---- load x tile: 512 tokens as 4 blocks of 128 on partitions
            x_sb = xpool.tile([128, NBLK, D], f32)
            nc.sync.dma_start(
                x_sb[:],
                x_d[t * TILE : (t + 1) * TILE, :].rearrange(
                    "(b p) d -> p b d", p=128
                ),
            )

            # ---- transpose to xT [d, tok] (PE), copy to SBUF (DVE)
            xt_ps = xt_ps_p.tile([128, TILE], f32)
            for b in range(NBLK):
                nc.tensor.transpose(
                    xt_ps[:, b * 128 : (b + 1) * 128],
                    x_sb[:, b, :],
                    id_sb[:],
                )
            xt_sb = xtpool.tile([128, TILE], f32r)
            nc.vector.tensor_copy(xt_sb[:], xt_ps[:])

            # ---- cT = A^T @ xT   (f32r, N=512)
            ct_ps = ct_ps_p.tile([128, TILE], f32)
            nc.tensor.matmul(
                ct_ps[:],
                a_sb[:],
                xt_sb[:],
                start=True,
                stop=True,
            )
            ct_sb = ctpool.tile([128, TILE], f32r)
            nc.scalar.copy(ct_sb[:], ct_ps[:])

            # ---- [v|r] = x @ [Wv|Wr] per block (f32r, N=256, normal layout)
            vr_ps = vr_ps_p.tile([128, NBLK, 2 * DP], f32)
            for b in range(NBLK):
                nc.tensor.matmul(
                    vr_ps[:, b, :],
                    xt_sb[:, b * 128 : (b + 1) * 128],
                    wvr_sb[:],
                    start=True,
                    stop=True,
                )
            vr_bf = vpool.tile([128, NBLK, 260], bf16)
            nc.gpsimd.memset(vr_bf[:, :, 0:1], 1.0)
            nc.scalar.copy(vr_bf[:, :, 1:257], vr_ps[:, :, :])

            # ---- scoresT per sample pair (f32r, full 128x128 blocks; the
            # cross-sample quadrants are garbage and never read by U)
            sc_ps = sc_ps_p.tile([128, NBLK, 2 * M], f32)
            for p in range(NBLK):
                c0 = p * 128
                nc.tensor.matmul(
                    sc_ps[:, p, :],
                    xt_sb[:, c0 : c0 + 128],
                    ct_sb[:, c0 : c0 + 128],
                    start=True,
                    stop=True,
                )
            exp_bf = epool.tile([128, NBLK, 2 * M], bf16)
            nc.scalar.activation(exp_bf[:], sc_ps[:], Exp)
            # zero the cross-sample quadrants -> block-diagonal stationary
            nc.gpsimd.memset(exp_bf[0:64, :, 64:128], 0.0)
            nc.gpsimd.memset(exp_bf[64:128, :, 0:64], 0.0)

            # ---- U = exp^T @ [v|1] per sample (bf16, diagonal-packed pairs)
            out_sb = opool.tile([128, NBLK, DP], f32)
            for h in range(2):
                u_ps = u_ps_p.tile([128, 2, 132], f32)
                for pp in range(2):
                    p = h * 2 + pp
                    nc.tensor.matmul(
                        u_ps[:, pp, 0:129],
                        exp_bf[:, p, :],
                        vr_bf[:, p, 0:129],
                        start=True,
                        stop=True,
                    )
                recip = rcpool.tile([128, 2, 1], f32)
                nc.vector.reciprocal(recip[:], u_ps[:, :, 0:1])
                for pp in range(2):
                    p = h * 2 + pp
                    # out = relu((U * (1/rowsum) + 0) + r)
                    nc.vector._custom_dve(
                        relu_affine,
                        out=out_sb[:, p, :],
                        in0=u_ps[:, pp, 1:129],
                        in1=vr_bf[:, p, 129:257],
                        s0=recip[:, pp, :],
                        s1=0.0,
                    )

            nc.sync.dma_start(
                out_d[t * TILE : (t + 1) * TILE, :].rearrange(
                    "(b p) d -> p b d", p=128
                ),
                out_sb[:],
            )

    nc.finalize()
    _BUILD_CACHE[key] = nc
    return nc


def make_inputs(x_shard, Wq, Wk, Wv, Wr):
    """Per-core input map from a token-flattened x shard [tok, D]."""
    A = (Wq.astype(np.float64) @ Wk.astype(np.float64).T).astype(np.float32)
    Wvr = np.concatenate([Wv, Wr], axis=1).astype(np.float32)
    ident = np.eye(128, dtype=np.float32)
    return {
        "x": np.ascontiguousarray(x_shard, dtype=np.float32),
        "Amat": A,
        "Wvr": Wvr,
        "ident": ident,
    }


def run(inputs, trace=False):
    """Run on 8 cores; returns (output [B,M,D], BassKernelResults)."""
    from concourse.bass_utils import run_bass_kernel_spmd

    x = np.asarray(inputs["x"], dtype=np.float32)
    Wq = np.asarray(inputs["Wq"], dtype=np.float32)
    Wk = np.asarray(inputs["Wk"], dtype=np.float32)
    Wv = np.asarray(inputs["Wv"], dtype=np.float32)
    Wr = np.asarray(inputs["Wr"], dtype=np.float32)

    nc = build()
    x_flat = x.reshape(NCORES, TOK, D)
    in_maps = [make_inputs(x_flat[i], Wq, Wk, Wv, Wr) for i in range(NCORES)]
    res = run_bass_kernel_spmd(nc, in_maps, list(range(NCORES)), trace=trace)
    out = np.stack([res.results[i]["out"] for i in range(NCORES)], axis=0)
    return out.reshape(B, M, DP), res


def kernel(x, Wq, Wk, Wv, Wr):
    out, _ = run({"x": x, "Wq": Wq, "Wk": Wk, "Wv": Wv, "Wr": Wr}, trace=False)
    return out
